# revision 11
# baseline (speedup 1.0000x reference)
"""Raw (non-Tile) Bass Block kernel for DiagonalMatrixModel, bf16-staged.

out = x * diagonal (column-broadcast scale).  Pure HBM-bandwidth problem:
the f32 version (32 MiB/core) sits at the DMA-engine roofline (~89 us),
so the host quantizes x (and diagonal) to bf16 before staging to device
DRAM, the device multiplies in bf16 and stores bf16, and the host
upcasts the result to f32.  Halves traffic to 16 MiB/core; rel-err from
the three bf16 roundings is ~3e-3, well inside the 2e-2 gate.

Measured model this revision is built around (NTFF traces, core 0):
  - The binding resource is the 16 SDMA engines (~25.5 GB/s each).  The
    partition->engine map is the fixed port swizzle
    engine = ((p>>2)&7)<<1 | ((p>>6)&1): engine e moves exactly the
    bytes of its 8 partitions.  exec ~= head (~8 us: 6 us runtime
    engine-boot preamble + ~2 us first-descriptor latency) + busiest
    engine's byte time + store receipt.
  - SDMA engine 15 (partitions 92-95 and 124-127) is ~19% slower per
    byte than the rest (380 vs 320 ns per 8 KiB descriptor, consistently,
    with or without SWDGE traffic).  With uniform layout every engine
    carries 1 MiB (64 x-rows); engines 0-14 finish in ~41 us and eng 15
    in ~49 us, setting the kernel's critical path.
  - Fix: assign partitions 92-95/124-127 only 6 x-rows each (48 rows ->
    ~36.6 us on eng 15) and park the 16 shaved rows as +1/+2 extra rows
    on partitions of the other engines via partition-strided DMAs
    (affine on both sides).  Busiest engine drops to ~42 us.
  - Per-DMA semaphore-update descriptors are uniform across engines and
    cheap (9 ns); keeping every DMA semmed is fine (the compiler
    requires sync info on DGE transfers anyway).

Dataflow:
  - Partition p holds x rows 8p..8p+7 as 8 column chunks of 4096
    (chunk k load = x[k::8, :], one 8 KiB descriptor per partition) -
    except the eng-15 partitions keep only rows 8p..8p+5, so chunks 6,7
    run as two partition-runs [0:92) and [96:124).  The 16 shaved rows
    load as four [4-partition-strided, 4096] "extra" DMAs into chunk-8
    columns of partitions {0,4,8,12},{16,20,24,28},{64,68,72,76},
    {96,100,104,108}.
  - diag [4096] bf16 -> SBUF [1,4096] (first DMA on the SP ring) -> PE
    ones-matmul broadcast -> PSUM f32 -> DVE cast-copies -> dtile
    [128,4096] bf16.
  - DVE multiplies chunk 8 (extras) first, then chunks 0..7, in place
    (full 128-partition ops; lanes holding no real data compute garbage
    that is never stored).
  - Stores mirror loads: SWDGE carries chunks 0-3 (writes mix into the
    read phase), SP ring carries G1,G2,4,6a,6b, ACT ring carries
    G3,G4,5,7a,7b; the small split-stores sit last so the tail transfer
    is short.
  - Bass-init head barrier / const memsets / block-end barrier stripped
    post-build; completion is guaranteed by SP's waits on the per-queue
    store semaphores.
"""

import numpy as np
import ml_dtypes

import concourse.bass as bass
import concourse.mybir as mybir
from concourse.bass_utils import run_bass_kernel_spmd

BATCH = 8192
SIZE = 4096
N_CORES = 8
ROWS = BATCH // N_CORES  # 1024
P = 128
RPP = ROWS // P  # 8 x-rows per partition in the uniform layout
MMN = 512  # one fp32 PSUM bank

# Engine-15 partitions (port swizzle ((p>>2)&7)<<1 | ((p>>6)&1) == 15).
E15A, E15B = 92, 124  # two runs of 4 partitions: [92:96), [124:128)
# Extra-row placements: four groups of 4 partitions, stride 4.
XGROUPS = [
    (742, 0),   # x rows 742,750,758,766 -> partitions 0,4,8,12
    (743, 16),  # x rows 743,751,759,767 -> partitions 16,20,24,28
    (998, 64),  # x rows 998,...         -> partitions 64,68,72,76
    (999, 96),  # x rows 999,...         -> partitions 96,100,104,108
]

_CACHE: dict = {}

BF16 = ml_dtypes.bfloat16


def _build() -> bass.Bass:
    nc = bass.Bass("TRN2", enable_asserts=False)
    bf = mybir.dt.bfloat16
    f32 = mybir.dt.float32
    x = nc.dram_tensor("x", [ROWS, SIZE], bf, kind="ExternalInput")
    dg = nc.dram_tensor("diagonal", [SIZE], bf, kind="ExternalInput")
    out = nc.dram_tensor("out", [ROWS, SIZE], bf, kind="ExternalOutput")

    xt = nc.alloc_sbuf_tensor("xt", [P, (RPP + 1) * SIZE], bf)
    diag1 = nc.alloc_sbuf_tensor("diag1", [1, SIZE], bf)
    ones = nc.alloc_sbuf_tensor("ones", [1, P], bf)
    dtile = nc.alloc_sbuf_tensor("dtile", [P, SIZE], bf)
    warm = nc.alloc_sbuf_tensor("warm", [1, P], bf)
    pt = [nc.alloc_psum_tensor(f"pt{j}", [P, MMN], f32) for j in range(SIZE // MMN)]

    def sb(k, p0=0, p1=P):  # SBUF chunk-k columns for partition range
        return xt[p0:p1, k * SIZE : (k + 1) * SIZE]

    def dr(t, k, p0=0, p1=P):  # DRAM rows {8p+k} for partition range
        return t[p0 * RPP + k : (p1 - 1) * RPP + k + 1 : RPP, :]

    def xsb(g):  # SBUF home of extra group g: chunk-8 cols, 4 parts stride 4
        return xt[XGROUPS[g][1] : XGROUPS[g][1] + 13 : 4, RPP * SIZE :]

    def xdr(t, g):  # DRAM rows of extra group g (stride 8 rows)
        r0 = XGROUPS[g][0]
        return t[r0 : r0 + 25 : RPP, :]

    # mul order: extras first, then chunks 0..7.  "mul of chunk k done"
    # == wait_ge(sem_mul, k+2); extras done == wait_ge(sem_mul, 1).
    from contextlib import ExitStack

    with ExitStack() as es, nc.Block(no_gpsimd_drain=True) as block:
        sem_diag = es.enter_context(nc.semaphore("sem_diag"))
        sem_ones = es.enter_context(nc.semaphore("sem_ones"))
        sem_mm = es.enter_context(nc.semaphore("sem_mm"))
        sem_mul = es.enter_context(nc.semaphore("sem_mul"))
        sem_cp = es.enter_context(nc.semaphore("sem_cp"))
        sem_warm = es.enter_context(nc.semaphore("sem_warm"))
        sem_ld = [es.enter_context(nc.semaphore(f"sem_ld{i}")) for i in range(RPP)]
        sem_xld = es.enter_context(nc.semaphore("sem_xld"))
        sem_stq = [es.enter_context(nc.semaphore(f"sem_stq{q}")) for q in range(3)]

        @block.sync
        def _(sync):
            sync.dma_start(
                out=diag1.ap(), in_=dg[:].partition_broadcast(1)
            ).then_inc(sem_diag, 16)
            for k in (0, 2, 4):  # even chunks load on SP ring
                sync.dma_start(out=sb(k), in_=dr(x, k)).then_inc(sem_ld[k], 16)
            sync.dma_start(out=sb(6, 0, E15A), in_=dr(x, 6, 0, E15A)).then_inc(
                sem_ld[6], 16
            )
            sync.dma_start(
                out=sb(6, E15A + 4, E15B), in_=dr(x, 6, E15A + 4, E15B)
            ).then_inc(sem_ld[6], 16)
            # Stores behind the loads: tiny extras first, small splits last.
            sync.wait_ge(sem_mul, 1)
            for g in (0, 1):
                sync.dma_start(out=xdr(out, g), in_=xsb(g)).then_inc(sem_stq[0], 16)
            sync.wait_ge(sem_mul, 6)
            sync.dma_start(out=dr(out, 4), in_=sb(4)).then_inc(sem_stq[0], 16)
            sync.wait_ge(sem_mul, 8)
            sync.dma_start(out=dr(out, 6, 0, E15A), in_=sb(6, 0, E15A)).then_inc(
                sem_stq[0], 16
            )
            sync.dma_start(
                out=dr(out, 6, E15A + 4, E15B), in_=sb(6, E15A + 4, E15B)
            ).then_inc(sem_stq[0], 16)
            # Kernel completion: all store queues drained.
            sync.wait_ge(sem_stq[0], 5 * 16)
            sync.wait_ge(sem_stq[1], 5 * 16)
            sync.wait_ge(sem_stq[2], 4 * 16)

        @block.scalar
        def _(act):
            for g in range(4):  # extra rows load first (tiny, gate mul #1)
                act.dma_start(out=xsb(g), in_=xdr(x, g)).then_inc(sem_xld, 16)
            for k in (1, 3, 5):  # odd chunks load on ACT ring
                act.dma_start(out=sb(k), in_=dr(x, k)).then_inc(sem_ld[k], 16)
            act.dma_start(out=sb(7, 0, E15A), in_=dr(x, 7, 0, E15A)).then_inc(
                sem_ld[7], 16
            )
            act.dma_start(
                out=sb(7, E15A + 4, E15B), in_=dr(x, 7, E15A + 4, E15B)
            ).then_inc(sem_ld[7], 16)
            act.wait_ge(sem_mul, 1)
            for g in (2, 3):
                act.dma_start(out=xdr(out, g), in_=xsb(g)).then_inc(sem_stq[1], 16)
            act.wait_ge(sem_mul, 7)
            act.dma_start(out=dr(out, 5), in_=sb(5)).then_inc(sem_stq[1], 16)
            act.wait_ge(sem_mul, 9)
            act.dma_start(out=dr(out, 7, 0, E15A), in_=sb(7, 0, E15A)).then_inc(
                sem_stq[1], 16
            )
            act.dma_start(
                out=dr(out, 7, E15A + 4, E15B), in_=sb(7, E15A + 4, E15B)
            ).then_inc(sem_stq[1], 16)

        @block.gpsimd
        def _(gp):
            # Early stores (chunks 0-3) ride SWDGE so writes mix into the
            # read phase on separate SDMA queue rows.  Warm-up DMA first:
            # Q7's first SWDGE op pays ~10us of setup.
            gp.dma_start(out=warm.ap(), in_=dg[0:P]).then_inc(sem_warm, 16)
            gp.wait_ge(sem_warm, 16)
            for k in range(4):
                gp.wait_ge(sem_mul, k + 2)
                gp.dma_start(out=dr(out, k), in_=sb(k)).then_inc(sem_stq[2], 16)

        @block.tensor
        def _(pe):
            pe.wait_ge(sem_ones, 1)
            pe.wait_ge(sem_diag, 16)
            for j in range(SIZE // MMN):
                pe.matmul(
                    out=pt[j].ap(),
                    lhsT=ones.ap(),
                    rhs=diag1.ap()[:, j * MMN : (j + 1) * MMN],
                    start=True,
                    stop=True,
                ).then_inc(sem_mm, 1)

        @block.vector
        def _(dve):
            dve.memset(ones.ap(), 1.0).then_inc(sem_ones, 1)
            for j in range(SIZE // MMN):
                dve.wait_ge(sem_mm, j + 1)
                dve.tensor_copy(
                    dtile.ap()[:, j * MMN : (j + 1) * MMN], pt[j].ap()
                ).then_inc(sem_cp, 1)
            dve.wait_ge(sem_cp, SIZE // MMN)
            dve.wait_ge(sem_xld, 4 * 16)  # extras (chunk 8) multiply first
            dve.tensor_mul(sb(RPP), sb(RPP), dtile.ap()).then_inc(sem_mul, 1)
            for k in range(RPP):
                dve.wait_ge(sem_ld[k], 32 if k >= 6 else 16)
                dve.tensor_mul(sb(k), sb(k), dtile.ap()).then_inc(sem_mul, 1)

    # Drop the Bass-init head barrier (drains + event-semaphores in the
    # preamble bb) and the const-AP memsets it protects — this kernel never
    # reads the const APs.  Every engine then starts its stream immediately
    # instead of waiting for the slowest engine to boot.  Also drop the
    # block-end barrier: kernel completion is already guaranteed by the SP
    # engine's final waits on the store-queue semaphores.
    blocks = nc.m.functions[0].blocks
    blocks[0].instructions = [
        inst
        for inst in blocks[0].instructions
        if type(inst).__name__ not in ("InstDrain", "InstEventSemaphore", "InstMemset")
    ]
    end_bb = blocks[-1]
    end_bb.instructions = [
        inst
        for inst in end_bb.instructions
        if type(inst).__name__ not in ("InstDrain", "InstEventSemaphore")
    ]
    return nc


def prep_in_maps(x: np.ndarray, diagonal: np.ndarray) -> list[dict]:
    """Host-side staging: quantize to bf16 and shard rows across cores."""
    xb = np.ascontiguousarray(np.asarray(x).astype(BF16))
    db = np.ascontiguousarray(np.asarray(diagonal).astype(BF16))
    shards = np.split(xb, N_CORES, axis=0)
    return [{"x": s, "diagonal": db} for s in shards]


def kernel(x: np.ndarray, diagonal: np.ndarray) -> np.ndarray:
    if "nc" not in _CACHE:
        _CACHE["nc"] = _build()
    nc = _CACHE["nc"]

    in_maps = prep_in_maps(x, diagonal)
    res = run_bass_kernel_spmd(nc, in_maps, list(range(N_CORES))).results
    return np.concatenate([r["out"] for r in res], axis=0).astype(np.float32)


# revision 12
# speedup vs baseline: 1.4775x; 1.4775x over previous
"""Raw (non-Tile) Bass Block kernel for DiagonalMatrixModel, bf16-staged.

out = x * diagonal (column-broadcast scale).  Pure HBM-bandwidth problem:
the f32 version (32 MiB/core) sits at the DMA-engine roofline (~89 us),
so the host quantizes x (and diagonal) to bf16 before staging to device
DRAM, the device multiplies in bf16 and stores bf16, and the host
upcasts the result to f32.  Halves traffic to 16 MiB/core; rel-err from
the three bf16 roundings is ~3e-3, well inside the 2e-2 gate.

Measured model this revision is built around (NTFF traces, core 0):
  - The binding resource is the 16 SDMA engines (~25.5 GB/s each).
    Descriptors are handed out in packets of 8, round-robin over engines
    RESTARTING AT ENGINE 0 FOR EVERY DMA (measured: DMAs with <16
    packets pile work onto the low engines; a 92-desc + 28-desc + 4-desc
    split overloaded engines 0-3 to 1.6 MiB and 76 us).  Hence every
    transfer here is exactly 128 descriptors = 16 packets so that all
    engines carry identical byte loads.
  - exec ~= head (~8 us: 6 us runtime engine-boot preamble + ~2 us
    first-descriptor latency) + busiest-engine byte time + receipt.
  - Engine 15 ran ~19% slower per 8 KiB descriptor in earlier uniform
    revisions (380 vs 320 ns); this revision doubles the descriptor
    size to 16 KiB (2-chunk DMAs) to amortize what looks like per-
    descriptor overhead, and to push per-engine throughput toward the
    ~27 GB/s asymptote.

Dataflow:
  - Per-core shard viewed as [128, 32768] (partition p = 8 consecutive
    x rows); view column c multiplies by diag[c mod 4096], so [128,4096]
    sub-chunks align exactly with the broadcast dtile.
  - diag [4096] bf16 -> SBUF [1,4096] (first DMA on the SP ring) -> PE
    ones-matmul broadcast -> PSUM f32 -> DVE cast-copies -> dtile
    [128,4096] bf16.
  - 4 double-chunk (2 MiB, 128 x 16 KiB descriptor) loads: SP ring takes
    D0 (cols 0:8192) and D2, ACT ring D1 and D3.  DVE multiplies the two
    4096-wide sub-chunks of each double-chunk in place as it lands.
  - 4 double-chunk stores: D0, D1 on SWDGE (softare-DGE emission is
    ~30 ns/descriptor, so a 2 MiB store costs ~4 us of Q7 time - fine
    for 2 stores, and the early stores mix writes into the read phase),
    D2 on the SP ring, D3 on the ACT ring behind their loads.
  - Bass-init head barrier / const memsets / block-end barrier stripped
    post-build; completion is guaranteed by SP's waits on the per-queue
    store semaphores.
"""

import numpy as np
import ml_dtypes

import concourse.bass as bass
import concourse.mybir as mybir
from concourse.bass_utils import run_bass_kernel_spmd

BATCH = 8192
SIZE = 4096
N_CORES = 8
ROWS = BATCH // N_CORES  # 1024
P = 128
RPP = ROWS // P  # 8 x-rows per partition
W = SIZE * RPP  # 32768 view columns
ND = 4  # double-chunks
DW = W // ND  # 8192 columns per double-chunk
MMN = 512  # one fp32 PSUM bank

_CACHE: dict = {}

BF16 = ml_dtypes.bfloat16


def _build() -> bass.Bass:
    nc = bass.Bass("TRN2", enable_asserts=False)
    bf = mybir.dt.bfloat16
    f32 = mybir.dt.float32
    x = nc.dram_tensor("x", [P, W], bf, kind="ExternalInput")
    dg = nc.dram_tensor("diagonal", [SIZE], bf, kind="ExternalInput")
    out = nc.dram_tensor("out", [P, W], bf, kind="ExternalOutput")

    xt = nc.alloc_sbuf_tensor("xt", [P, W], bf)
    diag1 = nc.alloc_sbuf_tensor("diag1", [1, SIZE], bf)
    ones = nc.alloc_sbuf_tensor("ones", [1, P], bf)
    dtile = nc.alloc_sbuf_tensor("dtile", [P, SIZE], bf)
    warm = nc.alloc_sbuf_tensor("warm", [1, P], bf)
    pt = [nc.alloc_psum_tensor(f"pt{j}", [P, MMN], f32) for j in range(SIZE // MMN)]

    def dchunk(t, j):  # double-chunk j: 8192 view columns, 16 KiB/partition
        return t[:, j * DW : (j + 1) * DW]

    def schunk(t, k):  # single 4096-column sub-chunk k (multiply unit)
        return t[:, k * SIZE : (k + 1) * SIZE]

    from contextlib import ExitStack

    with ExitStack() as es, nc.Block(no_gpsimd_drain=True) as block:
        sem_diag = es.enter_context(nc.semaphore("sem_diag"))
        sem_ones = es.enter_context(nc.semaphore("sem_ones"))
        sem_mm = es.enter_context(nc.semaphore("sem_mm"))
        sem_mul = es.enter_context(nc.semaphore("sem_mul"))
        sem_cp = es.enter_context(nc.semaphore("sem_cp"))
        sem_warm = es.enter_context(nc.semaphore("sem_warm"))
        sem_ld = [es.enter_context(nc.semaphore(f"sem_ld{j}")) for j in range(ND)]
        sem_stq = [es.enter_context(nc.semaphore(f"sem_stq{q}")) for q in range(3)]

        # sem_mul counts multiplies of 4096-wide sub-chunks in order:
        # double-chunk j is fully multiplied when sem_mul >= 2(j+1).
        @block.sync
        def _(sync):
            sync.dma_start(
                out=diag1.ap(), in_=dg[:].partition_broadcast(1)
            ).then_inc(sem_diag, 16)
            for j in (0, 2):  # D0, D2 load on SP ring
                sync.dma_start(out=dchunk(xt, j), in_=dchunk(x, j)).then_inc(
                    sem_ld[j], 16
                )
            sync.wait_ge(sem_mul, 6)  # D2 multiplied
            sync.dma_start(out=dchunk(out, 2), in_=dchunk(xt, 2)).then_inc(
                sem_stq[0], 16
            )
            # Kernel completion: all store queues drained.
            sync.wait_ge(sem_stq[0], 16)
            sync.wait_ge(sem_stq[1], 16)
            sync.wait_ge(sem_stq[2], 32)

        @block.scalar
        def _(act):
            for j in (1, 3):  # D1, D3 load on ACT ring
                act.dma_start(out=dchunk(xt, j), in_=dchunk(x, j)).then_inc(
                    sem_ld[j], 16
                )
            act.wait_ge(sem_mul, 8)  # D3 multiplied
            act.dma_start(out=dchunk(out, 3), in_=dchunk(xt, 3)).then_inc(
                sem_stq[1], 16
            )

        @block.gpsimd
        def _(gp):
            # Early stores (D0, D1) ride SWDGE so writes mix into the read
            # phase on separate SDMA queue rows.  Warm-up DMA first: Q7's
            # first SWDGE op pays ~10us of setup.
            gp.dma_start(out=warm.ap(), in_=dg[0:P]).then_inc(sem_warm, 16)
            gp.wait_ge(sem_warm, 16)
            for j in (0, 1):
                gp.wait_ge(sem_mul, 2 * (j + 1))
                gp.dma_start(out=dchunk(out, j), in_=dchunk(xt, j)).then_inc(
                    sem_stq[2], 16
                )

        @block.tensor
        def _(pe):
            pe.wait_ge(sem_ones, 1)
            pe.wait_ge(sem_diag, 16)
            for j in range(SIZE // MMN):
                pe.matmul(
                    out=pt[j].ap(),
                    lhsT=ones.ap(),
                    rhs=diag1.ap()[:, j * MMN : (j + 1) * MMN],
                    start=True,
                    stop=True,
                ).then_inc(sem_mm, 1)

        @block.vector
        def _(dve):
            dve.memset(ones.ap(), 1.0).then_inc(sem_ones, 1)
            for j in range(SIZE // MMN):
                dve.wait_ge(sem_mm, j + 1)
                dve.tensor_copy(
                    dtile.ap()[:, j * MMN : (j + 1) * MMN], pt[j].ap()
                ).then_inc(sem_cp, 1)
            dve.wait_ge(sem_cp, SIZE // MMN)
            for k in range(2 * ND):
                dve.wait_ge(sem_ld[k // 2], 16)
                dve.tensor_mul(schunk(xt, k), schunk(xt, k), dtile.ap()).then_inc(
                    sem_mul, 1
                )

    # Drop the Bass-init head barrier (drains + event-semaphores in the
    # preamble bb) and the const-AP memsets it protects — this kernel never
    # reads the const APs.  Every engine then starts its stream immediately
    # instead of waiting for the slowest engine to boot.  Also drop the
    # block-end barrier: kernel completion is already guaranteed by the SP
    # engine's final waits on the store-queue semaphores.
    blocks = nc.m.functions[0].blocks
    blocks[0].instructions = [
        inst
        for inst in blocks[0].instructions
        if type(inst).__name__ not in ("InstDrain", "InstEventSemaphore", "InstMemset")
    ]
    end_bb = blocks[-1]
    end_bb.instructions = [
        inst
        for inst in end_bb.instructions
        if type(inst).__name__ not in ("InstDrain", "InstEventSemaphore")
    ]
    return nc


def prep_in_maps(x: np.ndarray, diagonal: np.ndarray) -> list[dict]:
    """Host-side staging: quantize to bf16, shard rows, view as [128, W]."""
    xb = np.ascontiguousarray(np.asarray(x).astype(BF16))
    db = np.ascontiguousarray(np.asarray(diagonal).astype(BF16))
    shards = np.split(xb, N_CORES, axis=0)
    return [{"x": s.reshape(P, W), "diagonal": db} for s in shards]


def kernel(x: np.ndarray, diagonal: np.ndarray) -> np.ndarray:
    if "nc" not in _CACHE:
        _CACHE["nc"] = _build()
    nc = _CACHE["nc"]

    in_maps = prep_in_maps(x, diagonal)
    res = run_bass_kernel_spmd(nc, in_maps, list(range(N_CORES))).results
    full = np.concatenate([r["out"].reshape(ROWS, SIZE) for r in res], axis=0)
    return full.astype(np.float32)
